# revision 13
# baseline (speedup 1.0000x reference)
"""Trainium2 Bass kernel for AttentionFlowLayer (B=8, CS=1024, QS=128, D=1024).

Strategy: pure data-parallel over batch — core b computes batch b end to end,
no collectives.  Per core, the math is restructured to cut TensorEngine FLOPs:

  S[i,j] = ctx.w_c |i  +  q.w_q |j  +  (ctx*w_cq).q^T  (+ alpha_b, which cancels
           through both softmaxes and is dropped)
  Pn     = softmax_j(S)                               [CS, QS]
  qcw    = softmax_i(max_j S)                         [CS]
  ch[d]  = sum_i qcw[i] ctx[i,d]                      [D]
  out    = Pn @ (q @ b2T + 1 x bias)   (rank-QS factorization; bias row folded
         + (ctx . query_hatT) @ b3T     in via sum_j Pn[i,j] == 1)
         + (ctx . ch) @ b4T
  bias   = ch @ b1T + beta_b            (context_hat block is row-constant)

All matmuls run in bf16 with fp32 PSUM accumulation; softmax statistics in
fp32.  All four beta blocks stream through one fused output h-loop so the
TensorEngine never idles (keeps the HAM clock gate at 8/8).  Host-side, every
tensor is pre-packed so each DMA lands per-partition contiguous.  DMA traffic
is spread over the sync (context/out), gpsimd (beta streams), and scalar
queues so issue order never blocks arrival.
"""

import sys

sys.path.insert(0, "/opt/trn_rl_repo")

import numpy as np
import ml_dtypes

import concourse.bacc as bacc
import concourse.bass as bass
import concourse.mybir as mybir
import concourse.tile as tile
from concourse.bass_utils import run_bass_kernel_spmd

BF16 = mybir.dt.bfloat16
F32 = mybir.dt.float32
NPBF16 = ml_dtypes.bfloat16

B, CS, QS, D = 8, 1024, 128, 1024
H8 = 4 * D
NC = D // 128  # d-chunks
NT = CS // 128  # i-tiles
NH = H8 // 512  # o-chunks
ts = bass.ts

TRACE = False
_LAST_EXEC_NS = None
_NC_CACHE = None


def _build():
    nc = bacc.Bacc("TRN2", target_bir_lowering=False, debug=False)

    # [t, p, c, ii]: contextT tile per i-tile t -> SBUF [d128, c, i128]
    d_ct = nc.dram_tensor("ct8", [NT, 128, NC, 128], BF16, kind="ExternalInput")
    # [p, t, d]: context natural, i on partitions
    d_cx = nc.dram_tensor("cx8", [128, NT, D], BF16, kind="ExternalInput")
    d_q = nc.dram_tensor("q", [QS, D], BF16, kind="ExternalInput")
    # [p, c, j]: queryT, d on partitions
    d_qT = nc.dram_tensor("qT8", [128, NC, QS], BF16, kind="ExternalInput")
    d_wc = nc.dram_tensor("wcb8", [128, NC], BF16, kind="ExternalInput")
    d_wq = nc.dram_tensor("wqb8", [128, NC], BF16, kind="ExternalInput")
    d_wcq = nc.dram_tensor("wcq8", [128, NC], F32, kind="ExternalInput")
    d_id = nc.dram_tensor("identf", [128, 128], F32, kind="ExternalInput")
    # beta blocks pre-packed per output-chunk h, per-partition contiguous:
    # d_b1/d_b2: [h, p, c, f];  d_b34: [h, p, g, f] g: 0..7 -> b3, 8..15 -> b4
    d_b1 = nc.dram_tensor("b1h", [NH, 128, NC, 512], BF16, kind="ExternalInput")
    d_b2 = nc.dram_tensor("b2h", [NH, 128, NC, 512], BF16, kind="ExternalInput")
    d_b34 = nc.dram_tensor("b34h", [NH, 128, 2 * NC, 512], BF16, kind="ExternalInput")
    d_bb = nc.dram_tensor("bb", [1, H8], BF16, kind="ExternalInput")
    d_out = nc.dram_tensor("out", [CS, H8], F32, kind="ExternalOutput")

    with tile.TileContext(nc) as tc:
        with tc.tile_pool(name="persist", bufs=1) as pp:
            # ---- persistent SBUF tensors -------------------------------
            CTt = [
                pp.tile([128, NC, 128], BF16, tag=f"ct{t}", name=f"CTt{t}")
                for t in range(NT)
            ]
            CX = pp.tile([128, NT, D], BF16)        # context natural [i128, t, d]
            B3T = pp.tile([128, NC, CS], BF16)      # (ctx * query_hat)^T [d, c, i]
            PnT = pp.tile([128, CS], BF16)          # softmax_j(S)^T  [j, i]
            Q = pp.tile([QS, D], BF16)              # query natural   [j, d]
            QT = pp.tile([128, NC, QS], BF16)       # queryT          [d, j]
            QSC = pp.tile([128, NC, QS], BF16)      # queryT * w_cq[d]
            WCb = pp.tile([128, NC], BF16)
            WQb = pp.tile([128, NC], BF16)
            WCQ = pp.tile([128, NC], F32)
            IDENT = pp.tile([128, 128], F32)
            BBr = pp.tile([1, NH, 512], BF16)
            CWR = pp.tile([1, NT, 128], BF16)       # ctx . w_c as a row
            QWB = pp.tile([1, QS], BF16)            # q . w_q as a row
            ONESb = pp.tile([1, 128], BF16)
            ONESC = pp.tile([128, 1], F32)
            ONESR = pp.tile([1, 128], F32)
            NEGMX = pp.tile([128, NT], F32)         # -max_j S, per i-tile col
            SM = pp.tile([128, NT], F32)
            RSM = pp.tile([128, NT], F32)
            ECOL = pp.tile([128, NT], F32)          # exp(mx)
            TOT = pp.tile([1, 1], F32)
            RTOT = pp.tile([1, 1], F32)
            RTOTB = pp.tile([128, 1], F32)
            QCWC = pp.tile([128, NT], BF16)         # qcw columns
            CH = pp.tile([128, NC], F32)            # context_hat columns
            CHb = pp.tile([128, NC], BF16)

            # ---- loads: big streams on sync in consumption order, small
            # consts on the gpsimd queue (ahead of its beta fetches) ----
            nc.gpsimd.dma_start(WCQ[:], d_wcq[:])
            nc.gpsimd.dma_start(WCb[:], d_wc[:])
            nc.gpsimd.dma_start(WQb[:], d_wq[:])
            nc.gpsimd.dma_start(IDENT[:], d_id[:])
            nc.gpsimd.dma_start(BBr[:], d_bb[:].rearrange("o (h f) -> o h f", f=512))
            nc.sync.dma_start(QT[:], d_qT[:])
            for t in range(NT):
                nc.sync.dma_start(CTt[t][:], d_ct[t])
            nc.sync.dma_start(Q[:], d_q[:])
            nc.sync.dma_start(CX[:], d_cx[:])
            nc.vector.memset(ONESb[:], 1.0)
            nc.vector.memset(ONESC[:], 1.0)
            nc.vector.memset(ONESR[:], 1.0)

            for c in range(NC):
                nc.vector.tensor_scalar_mul(QSC[:, c, :], QT[:, c, :], WCQ[:, c : c + 1])

            # ---- per-tile: cw row chunk, S, softmax_j, PnT ------------
            with (
                tc.tile_pool(name="pbq", bufs=1, space="PSUM") as pbq,
                tc.tile_pool(name="pbc", bufs=2, space="PSUM") as pbc,
                tc.tile_pool(name="ps", bufs=3, space="PSUM") as ps,
                tc.tile_pool(name="pt", bufs=2, space="PSUM") as pt,
                tc.tile_pool(name="sp", bufs=2) as sp,
            ):
                PS_qw = pbq.tile([1, QS], F32)
                for c in range(NC):
                    nc.tensor.matmul(
                        PS_qw[:], WQb[:, c : c + 1], QT[:, c, :],
                        start=(c == 0), stop=(c == NC - 1),
                    )
                nc.vector.tensor_copy(QWB[:], PS_qw[:])

                for t in range(NT):
                    PS_cwt = pbc.tile([1, 128], F32, tag="cw")
                    for c in range(NC):
                        nc.tensor.matmul(
                            PS_cwt[:], WCb[:, c : c + 1], CTt[t][:, c, :],
                            start=(c == 0), stop=(c == NC - 1),
                        )
                    nc.vector.tensor_copy(CWR[:, t, :], PS_cwt[:])

                    PS_S = ps.tile([128, QS], F32, tag="s")
                    for c in range(NC):
                        nc.tensor.matmul(
                            PS_S[:], CTt[t][:, c, :], QSC[:, c, :],
                            start=(c == 0), stop=False,
                        )
                    nc.tensor.matmul(PS_S[:], ONESb[:], QWB[:], start=False, stop=False)
                    nc.tensor.matmul(
                        PS_S[:], CWR[:, t, :], ONESb[:], start=False, stop=True
                    )
                    nc.vector.tensor_reduce(
                        NEGMX[:, t : t + 1], PS_S[:],
                        axis=mybir.AxisListType.X, op=mybir.AluOpType.max, negate=True,
                    )
                    P_sb = sp.tile([128, QS], F32, tag="p")
                    nc.scalar.activation(
                        P_sb[:], PS_S[:], mybir.ActivationFunctionType.Exp,
                        bias=NEGMX[:, t : t + 1], accum_out=SM[:, t : t + 1],
                    )
                    nc.vector.reciprocal(RSM[:, t : t + 1], SM[:, t : t + 1])
                    Pn_sb = sp.tile([128, QS], F32, tag="pn")
                    nc.vector.tensor_scalar_mul(Pn_sb[:], P_sb[:], RSM[:, t : t + 1])
                    PS_T = pt.tile([128, 128], F32, tag="t")
                    nc.tensor.transpose(PS_T[:], Pn_sb[:], IDENT[:])
                    nc.vector.tensor_copy(PnT[:, ts(t, 128)], PS_T[:])

            # ---- softmax_i(max_j S) -> qcw, context_hat ---------------
            # The qh/B3T matmuls are interleaved into the (DVE-latency-
            # bound) qcw chain to keep the PE busy.
            nc.scalar.activation(
                ECOL[:], NEGMX[:], mybir.ActivationFunctionType.Exp, scale=-1.0
            )
            with (
                tc.tile_pool(name="pd", bufs=1, space="PSUM") as pd,
                tc.tile_pool(name="pg", bufs=2, space="PSUM") as pg,
            ):
                PS_tot = pd.tile([1, NT], F32)
                nc.tensor.matmul(PS_tot[:], ONESC[:], ECOL[:])
                nc.vector.tensor_reduce(
                    TOT[:], PS_tot[:], axis=mybir.AxisListType.X, op=mybir.AluOpType.add
                )
                nc.vector.reciprocal(RTOT[:], TOT[:])

                for hh in range(2):
                    for c in range(NC):
                        PS_qh = pg.tile([128, 512], F32, tag="qh")
                        nc.tensor.matmul(
                            PS_qh[:], Q[:, ts(c, 128)], PnT[:, hh * 512 : (hh + 1) * 512]
                        )
                        for tt in range(4):
                            t = hh * 4 + tt
                            nc.vector.tensor_tensor(
                                B3T[:, c, ts(t, 128)],
                                CTt[t][:, c, :],
                                PS_qh[:, ts(tt, 128)],
                                op=mybir.AluOpType.mult,
                            )

                PS_rb = pd.tile([128, 1], F32)
                nc.tensor.matmul(PS_rb[:], ONESR[:], RTOT[:])
                nc.vector.tensor_copy(RTOTB[:], PS_rb[:])
                nc.vector.tensor_scalar_mul(QCWC[:], ECOL[:], RTOTB[:])

            with tc.tile_pool(name="pe", bufs=1, space="PSUM") as pe:
                PS_ch = pe.tile([128, NC], F32)
                for t in range(NT):
                    for c in range(NC):
                        nc.tensor.matmul(
                            PS_ch[:, c : c + 1], CX[:, t, ts(c, 128)], QCWC[:, t : t + 1],
                            start=(t == 0 and c == 0), stop=(t == NT - 1 and c == NC - 1),
                            skip_group_check=True,
                        )
                nc.vector.tensor_copy(CH[:], PS_ch[:])
                nc.vector.tensor_copy(CHb[:], PS_ch[:])

            # ---- fused output loop ------------------------------------
            with (
                tc.tile_pool(name="pw", bufs=1, space="PSUM") as pw,
                tc.tile_pool(name="pi", bufs=1, space="PSUM") as pi,
                tc.tile_pool(name="pj", bufs=6, space="PSUM") as pj,
                tc.tile_pool(name="bwp", bufs=2) as bwp,
                tc.tile_pool(name="whp", bufs=3) as whp,
                tc.tile_pool(name="op", bufs=6) as outp,
            ):
                def fetch_beta(h):
                    BW34 = bwp.tile([128, 2 * NC, 512], BF16, tag="bw34", name="BW34")
                    nc.gpsimd.dma_start(BW34[:], d_b34[h])
                    BWB = bwp.tile([128, NC, 512], BF16, tag="bwb", name="BWB")
                    nc.gpsimd.dma_start(BWB[:], d_b1[h])
                    BW2 = bwp.tile([128, NC, 512], BF16, tag="bw2", name="BW2")
                    nc.gpsimd.dma_start(BW2[:], d_b2[h])
                    return BW34, BWB, BW2

                def produce_w2h(h, BWB, BW2):
                    # bias_h = ch @ b1T|h + beta_b|h, then W2H = q @ b2T|h
                    # + ones x bias_h  (consumed via Pn whose rows sum to 1).
                    PS_b = pi.tile([1, 512], F32, tag="bi", name="PS_b")
                    for c in range(NC):
                        nc.tensor.matmul(
                            PS_b[:], CHb[:, c : c + 1], BWB[:, c, :],
                            start=(c == 0), stop=False,
                        )
                    nc.tensor.matmul(
                        PS_b[:], ONESb[:, 0:1], BBr[:, h, :], start=False, stop=True
                    )
                    BIH = whp.tile([1, 512], BF16, tag="bih", name="BIH")
                    nc.vector.tensor_copy(BIH[:], PS_b[:])
                    PS_w2 = pw.tile([128, 512], F32, tag="w2", name="PS_w2")
                    for c in range(NC):
                        nc.tensor.matmul(
                            PS_w2[:], QT[:, c, :], BW2[:, c, :],
                            start=(c == 0), stop=False,
                        )
                    nc.tensor.matmul(
                        PS_w2[:], ONESb[:], BIH[:], start=False, stop=True
                    )
                    W2H = whp.tile([128, 512], BF16, tag="w2h", name="W2H")
                    nc.vector.tensor_copy(W2H[:], PS_w2[:])
                    return W2H

                def scale_b4(BW34):
                    # Fold diag(ch) into the streamed b4 tiles in place:
                    # (ctx . ch) @ b4T == ctx @ (diag(ch) b4T).
                    for c in range(NC):
                        nc.vector.tensor_scalar_mul(
                            BW34[:, NC + c, :], BW34[:, NC + c, :], CH[:, c : c + 1]
                        )

                beta_cur = fetch_beta(0)
                scale_b4(beta_cur[0])
                w2h_cur = produce_w2h(0, beta_cur[1], beta_cur[2])
                for h in range(NH):
                    BW34 = beta_cur[0]
                    beta_next = fetch_beta(h + 1) if h + 1 < NH else None
                    for t in range(NT):
                        PS_o = pj.tile([128, 512], F32, tag="o", name="PS_o")
                        for c in range(NC):
                            nc.tensor.matmul(
                                PS_o[:], B3T[:, c, ts(t, 128)], BW34[:, c, :],
                                start=(c == 0), stop=False,
                            )
                        for c in range(NC):
                            nc.tensor.matmul(
                                PS_o[:], CTt[t][:, c, :], BW34[:, NC + c, :],
                                start=False, stop=False,
                            )
                        nc.tensor.matmul(
                            PS_o[:], PnT[:, ts(t, 128)], w2h_cur[:],
                            start=False, stop=True,
                        )
                        OS = outp.tile([128, 512], F32, tag="os", name="OS")
                        if t % 2 == 0:
                            nc.vector.tensor_copy(OS[:], PS_o[:])
                        else:
                            nc.scalar.copy(OS[:], PS_o[:])
                        nc.sync.dma_start(d_out[ts(t, 128), ts(h, 512)], OS[:])
                    if beta_next is not None:
                        scale_b4(beta_next[0])
                        w2h_cur = produce_w2h(h + 1, beta_next[1], beta_next[2])
                        beta_cur = beta_next

    nc.compile()
    return nc


def _get_nc():
    global _NC_CACHE
    if _NC_CACHE is None:
        _NC_CACHE = _build()
    return _NC_CACHE


def _prep_shared(alpha_w, beta_w, beta_b):
    wc, wq, wcq = alpha_w[:D], alpha_w[D : 2 * D], alpha_w[2 * D :]
    shared = {
        "wcb8": np.ascontiguousarray(wc.reshape(NC, 128).T).astype(NPBF16),
        "wqb8": np.ascontiguousarray(wq.reshape(NC, 128).T).astype(NPBF16),
        "wcq8": np.ascontiguousarray(wcq.reshape(NC, 128).T).astype(np.float32),
        "identf": np.eye(128, dtype=np.float32),
        "bb": beta_b.reshape(1, H8).astype(NPBF16),
    }
    betaT = np.ascontiguousarray(beta_w.T)  # [f, o]
    # [h, p, c, f] per-partition contiguous packs of each block
    shared["b1h"] = np.ascontiguousarray(
        betaT[0:D].reshape(NC, 128, NH, 512).transpose(2, 1, 0, 3)
    ).astype(NPBF16)
    shared["b2h"] = np.ascontiguousarray(
        betaT[D : 2 * D].reshape(NC, 128, NH, 512).transpose(2, 1, 0, 3)
    ).astype(NPBF16)
    shared["b34h"] = np.ascontiguousarray(
        betaT[2 * D : 4 * D].reshape(2 * NC, 128, NH, 512).transpose(2, 1, 0, 3)
    ).astype(NPBF16)
    return shared


def kernel(context, query, alpha_w, alpha_b, beta_w, beta_b):
    global _LAST_EXEC_NS
    context = np.asarray(context, dtype=np.float32)
    query = np.asarray(query, dtype=np.float32)
    alpha_w = np.asarray(alpha_w, dtype=np.float32)
    beta_w = np.asarray(beta_w, dtype=np.float32)
    beta_b = np.asarray(beta_b, dtype=np.float32)

    shared = _prep_shared(alpha_w, beta_w, beta_b)

    in_maps = []
    for b in range(B):
        cb = context[b]
        qb = query[b]
        m = {
            # [t, ii, c, p] -> [t, p, c, ii]
            "ct8": np.ascontiguousarray(
                cb.reshape(NT, 128, NC, 128).transpose(0, 3, 2, 1)
            ).astype(NPBF16),
            # [t, ii, d] -> [ii, t, d]
            "cx8": np.ascontiguousarray(
                cb.reshape(NT, 128, D).transpose(1, 0, 2)
            ).astype(NPBF16),
            "q": qb.astype(NPBF16),
            # qT [d, j]: [c, p, j] -> [p, c, j]
            "qT8": np.ascontiguousarray(
                qb.T.reshape(NC, 128, QS).transpose(1, 0, 2)
            ).astype(NPBF16),
        }
        m.update(shared)
        in_maps.append(m)

    nc = _get_nc()
    res = run_bass_kernel_spmd(nc, in_maps, list(range(B)), trace=TRACE)
    _LAST_EXEC_NS = res.exec_time_ns
    out = np.stack([res.results[b]["out"] for b in range(B)], axis=0)
    return out


# revision 14
# speedup vs baseline: 1.1975x; 1.1975x over previous
"""Trainium2 Bass kernel for AttentionFlowLayer (B=8, CS=1024, QS=128, D=1024).

Strategy: pure data-parallel over batch — core b computes batch b end to end,
no collectives.  Per core, the math is restructured to cut TensorEngine FLOPs:

  S[i,j] = ctx.w_c |i  +  q.w_q |j  +  (ctx*w_cq).q^T  (+ alpha_b, which cancels
           through both softmaxes and is dropped)
  Pn     = softmax_j(S)                               [CS, QS]
  qcw    = softmax_i(max_j S)                         [CS]
  ch[d]  = sum_i qcw[i] ctx[i,d]                      [D]
  out    = Pn @ (q @ b2T + 1 x bias)   (rank-QS factorization; bias row folded
         + (ctx . query_hatT) @ b3T     in via sum_j Pn[i,j] == 1)
         + (ctx . ch) @ b4T
  bias   = ch @ b1T + beta_b            (context_hat block is row-constant)

All matmuls run in bf16 with fp32 PSUM accumulation; softmax statistics in
fp32.  All four beta blocks stream through one fused output h-loop so the
TensorEngine never idles (keeps the HAM clock gate at 8/8).  Host-side, every
tensor is pre-packed so each DMA lands per-partition contiguous.  DMA traffic
is spread over the sync (context/out), gpsimd (beta streams), and scalar
queues so issue order never blocks arrival.
"""

import sys

sys.path.insert(0, "/opt/trn_rl_repo")

import numpy as np
import ml_dtypes

import concourse.bacc as bacc
import concourse.bass as bass
import concourse.mybir as mybir
import concourse.tile as tile
from concourse.bass_utils import run_bass_kernel_spmd

BF16 = mybir.dt.bfloat16
F32 = mybir.dt.float32
NPBF16 = ml_dtypes.bfloat16

B, CS, QS, D = 8, 1024, 128, 1024
H8 = 4 * D
NC = D // 128  # d-chunks
NT = CS // 128  # i-tiles
NH = H8 // 512  # o-chunks
ts = bass.ts

TRACE = False
_LAST_EXEC_NS = None
_NC_CACHE = None


def _build():
    nc = bacc.Bacc("TRN2", target_bir_lowering=False, debug=False)

    # [t, p, c, ii]: contextT tile per i-tile t -> SBUF [d128, c, i128]
    d_ct = nc.dram_tensor("ct8", [NT, 128, NC, 128], BF16, kind="ExternalInput")
    # [p, t, d]: context natural, i on partitions
    d_cx = nc.dram_tensor("cx8", [128, NT, D], BF16, kind="ExternalInput")
    d_q = nc.dram_tensor("q", [QS, D], BF16, kind="ExternalInput")
    # [p, c, j]: queryT, d on partitions
    d_qT = nc.dram_tensor("qT8", [128, NC, QS], BF16, kind="ExternalInput")
    d_wc = nc.dram_tensor("wcb8", [128, NC], BF16, kind="ExternalInput")
    d_wq = nc.dram_tensor("wqb8", [128, NC], BF16, kind="ExternalInput")
    d_wcq = nc.dram_tensor("wcq8", [128, NC], F32, kind="ExternalInput")
    d_id = nc.dram_tensor("identf", [128, 128], F32, kind="ExternalInput")
    # beta blocks pre-packed per output-chunk h, per-partition contiguous:
    # d_b1/d_b2: [h, p, c, f];  d_b34: [h, p, g, f] g: 0..7 -> b3, 8..15 -> b4
    d_b1 = nc.dram_tensor("b1h", [NH, 128, NC, 512], BF16, kind="ExternalInput")
    d_b2 = nc.dram_tensor("b2h", [NH, 128, NC, 512], BF16, kind="ExternalInput")
    d_b34 = nc.dram_tensor("b34h", [NH, 128, 2 * NC, 512], BF16, kind="ExternalInput")
    d_bb = nc.dram_tensor("bb", [1, H8], BF16, kind="ExternalInput")
    d_out = nc.dram_tensor("out", [CS, H8], F32, kind="ExternalOutput")

    with tile.TileContext(nc) as tc:
        with tc.tile_pool(name="persist", bufs=1) as pp:
            # ---- persistent SBUF tensors -------------------------------
            CTt = [
                pp.tile([128, NC, 128], BF16, tag=f"ct{t}", name=f"CTt{t}")
                for t in range(NT)
            ]
            CX = pp.tile([128, NT, D], BF16)        # context natural [i128, t, d]
            B3T = pp.tile([128, NC, CS], BF16)      # (ctx * query_hat)^T [d, c, i]
            PnT = pp.tile([128, CS], BF16)          # softmax_j(S)^T  [j, i]
            Q = pp.tile([QS, D], BF16)              # query natural   [j, d]
            QT = pp.tile([128, NC, QS], BF16)       # queryT          [d, j]
            QSC = pp.tile([128, NC, QS], BF16)      # queryT * w_cq[d]
            WCb = pp.tile([128, NC], BF16)
            WQb = pp.tile([128, NC], BF16)
            WCQ = pp.tile([128, NC], F32)
            IDENT = pp.tile([128, 128], F32)
            BBr = pp.tile([1, NH, 512], BF16)
            CWR = pp.tile([1, NT, 128], BF16)       # ctx . w_c as a row
            QWB = pp.tile([1, QS], BF16)            # q . w_q as a row
            ONESb = pp.tile([1, 128], BF16)
            ONESC = pp.tile([128, 1], F32)
            ONESR = pp.tile([1, 128], F32)
            NEGMX = pp.tile([128, NT], F32)         # -max_j S, per i-tile col
            SM = pp.tile([128, NT], F32)
            RSM = pp.tile([128, NT], F32)
            ECOL = pp.tile([128, NT], F32)          # exp(mx)
            TOT = pp.tile([1, 1], F32)
            RTOT = pp.tile([1, 1], F32)
            RTOTB = pp.tile([128, 1], F32)
            QCWC = pp.tile([128, NT], BF16)         # qcw columns
            CH = pp.tile([128, NC], F32)            # context_hat columns
            CHb = pp.tile([128, NC], BF16)

            # ---- loads: big streams on sync in consumption order, small
            # consts on the gpsimd queue (ahead of its beta fetches) ----
            nc.gpsimd.dma_start(WCQ[:], d_wcq[:])
            nc.gpsimd.dma_start(WCb[:], d_wc[:])
            nc.gpsimd.dma_start(WQb[:], d_wq[:])
            nc.gpsimd.dma_start(IDENT[:], d_id[:])
            nc.gpsimd.dma_start(BBr[:], d_bb[:].rearrange("o (h f) -> o h f", f=512))
            nc.sync.dma_start(QT[:], d_qT[:])
            for t in range(NT):
                nc.sync.dma_start(CTt[t][:], d_ct[t])
            nc.sync.dma_start(Q[:], d_q[:])
            nc.sync.dma_start(CX[:], d_cx[:])
            nc.vector.memset(ONESb[:], 1.0)
            nc.vector.memset(ONESC[:], 1.0)
            nc.vector.memset(ONESR[:], 1.0)

            for c in range(NC):
                nc.vector.tensor_scalar_mul(QSC[:, c, :], QT[:, c, :], WCQ[:, c : c + 1])

            # ---- per-tile: cw row chunk, S, softmax_j, PnT ------------
            with (
                tc.tile_pool(name="pbq", bufs=1, space="PSUM") as pbq,
                tc.tile_pool(name="pbc", bufs=2, space="PSUM") as pbc,
                tc.tile_pool(name="ps", bufs=3, space="PSUM") as ps,
                tc.tile_pool(name="pt", bufs=2, space="PSUM") as pt,
                tc.tile_pool(name="sp", bufs=2) as sp,
            ):
                PS_qw = pbq.tile([1, QS], F32)
                for c in range(NC):
                    nc.tensor.matmul(
                        PS_qw[:], WQb[:, c : c + 1], QT[:, c, :],
                        start=(c == 0), stop=(c == NC - 1),
                    )
                nc.vector.tensor_copy(QWB[:], PS_qw[:])

                for t in range(NT):
                    PS_cwt = pbc.tile([1, 128], F32, tag="cw")
                    for c in range(NC):
                        nc.tensor.matmul(
                            PS_cwt[:], WCb[:, c : c + 1], CTt[t][:, c, :],
                            start=(c == 0), stop=(c == NC - 1),
                        )
                    nc.vector.tensor_copy(CWR[:, t, :], PS_cwt[:])

                    PS_S = ps.tile([128, QS], F32, tag="s")
                    for c in range(NC):
                        nc.tensor.matmul(
                            PS_S[:], CTt[t][:, c, :], QSC[:, c, :],
                            start=(c == 0), stop=False,
                        )
                    nc.tensor.matmul(PS_S[:], ONESb[:], QWB[:], start=False, stop=False)
                    nc.tensor.matmul(
                        PS_S[:], CWR[:, t, :], ONESb[:], start=False, stop=True
                    )
                    nc.vector.tensor_reduce(
                        NEGMX[:, t : t + 1], PS_S[:],
                        axis=mybir.AxisListType.X, op=mybir.AluOpType.max, negate=True,
                    )
                    P_sb = sp.tile([128, QS], F32, tag="p")
                    nc.scalar.activation(
                        P_sb[:], PS_S[:], mybir.ActivationFunctionType.Exp,
                        bias=NEGMX[:, t : t + 1], accum_out=SM[:, t : t + 1],
                    )
                    nc.vector.reciprocal(RSM[:, t : t + 1], SM[:, t : t + 1])
                    Pn_sb = sp.tile([128, QS], F32, tag="pn")
                    nc.vector.tensor_scalar_mul(Pn_sb[:], P_sb[:], RSM[:, t : t + 1])
                    PS_T = pt.tile([128, 128], F32, tag="t")
                    nc.tensor.transpose(PS_T[:], Pn_sb[:], IDENT[:])
                    nc.vector.tensor_copy(PnT[:, ts(t, 128)], PS_T[:])

            # ---- softmax_i(max_j S) -> qcw, context_hat ---------------
            # The qh/B3T matmuls are interleaved into the (DVE-latency-
            # bound) qcw chain to keep the PE busy.
            nc.scalar.activation(
                ECOL[:], NEGMX[:], mybir.ActivationFunctionType.Exp, scale=-1.0
            )
            with (
                tc.tile_pool(name="pd", bufs=1, space="PSUM") as pd,
                tc.tile_pool(name="pg", bufs=2, space="PSUM") as pg,
            ):
                PS_tot = pd.tile([1, NT], F32)
                nc.tensor.matmul(PS_tot[:], ONESC[:], ECOL[:])
                nc.vector.tensor_reduce(
                    TOT[:], PS_tot[:], axis=mybir.AxisListType.X, op=mybir.AluOpType.add
                )
                nc.vector.reciprocal(RTOT[:], TOT[:])

                for hh in range(2):
                    for c in range(NC):
                        PS_qh = pg.tile([128, 512], F32, tag="qh")
                        nc.tensor.matmul(
                            PS_qh[:], Q[:, ts(c, 128)], PnT[:, hh * 512 : (hh + 1) * 512]
                        )
                        for tt in range(4):
                            t = hh * 4 + tt
                            nc.vector.tensor_tensor(
                                B3T[:, c, ts(t, 128)],
                                CTt[t][:, c, :],
                                PS_qh[:, ts(tt, 128)],
                                op=mybir.AluOpType.mult,
                            )

                PS_rb = pd.tile([128, 1], F32)
                nc.tensor.matmul(PS_rb[:], ONESR[:], RTOT[:])
                nc.vector.tensor_copy(RTOTB[:], PS_rb[:])
                nc.vector.tensor_scalar_mul(QCWC[:], ECOL[:], RTOTB[:])

            with tc.tile_pool(name="pe", bufs=1, space="PSUM") as pe:
                PS_ch = pe.tile([128, NC], F32)
                for t in range(NT):
                    for c in range(NC):
                        nc.tensor.matmul(
                            PS_ch[:, c : c + 1], CX[:, t, ts(c, 128)], QCWC[:, t : t + 1],
                            start=(t == 0 and c == 0), stop=(t == NT - 1 and c == NC - 1),
                            skip_group_check=True,
                        )
                nc.vector.tensor_copy(CH[:], PS_ch[:])
                nc.vector.tensor_copy(CHb[:], PS_ch[:])

            # ---- fused output loop ------------------------------------
            with (
                tc.tile_pool(name="pw", bufs=1, space="PSUM") as pw,
                tc.tile_pool(name="pi", bufs=1, space="PSUM") as pi,
                tc.tile_pool(name="pj", bufs=5, space="PSUM") as pj,
                tc.tile_pool(name="bwp", bufs=2) as bwp,
                tc.tile_pool(name="whp", bufs=2) as whp,
                tc.tile_pool(name="op", bufs=4) as outp,
            ):
                def fetch_beta(h):
                    BW34 = bwp.tile([128, 2 * NC, 512], BF16, tag="bw34", name="BW34")
                    nc.gpsimd.dma_start(BW34[:], d_b34[h])
                    BWB = bwp.tile([128, NC, 512], BF16, tag="bwb", name="BWB")
                    nc.gpsimd.dma_start(BWB[:], d_b1[h])
                    BW2 = bwp.tile([128, NC, 512], BF16, tag="bw2", name="BW2")
                    nc.gpsimd.dma_start(BW2[:], d_b2[h])
                    return BW34, BWB, BW2

                def produce_w2h(h, BWB, BW2):
                    # bias_h = ch @ b1T|h + beta_b|h, then W2H = q @ b2T|h
                    # + ones x bias_h  (consumed via Pn whose rows sum to 1).
                    PS_b = pi.tile([1, 512], F32, tag="bi", name="PS_b")
                    for c in range(NC):
                        nc.tensor.matmul(
                            PS_b[:], CHb[:, c : c + 1], BWB[:, c, :],
                            start=(c == 0), stop=False,
                        )
                    nc.tensor.matmul(
                        PS_b[:], ONESb[:, 0:1], BBr[:, h, :], start=False, stop=True
                    )
                    BIH = whp.tile([1, 512], BF16, tag="bih", name="BIH")
                    nc.vector.tensor_copy(BIH[:], PS_b[:])
                    PS_w2 = pw.tile([128, 512], F32, tag="w2", name="PS_w2")
                    for c in range(NC):
                        nc.tensor.matmul(
                            PS_w2[:], QT[:, c, :], BW2[:, c, :],
                            start=(c == 0), stop=False,
                        )
                    nc.tensor.matmul(
                        PS_w2[:], ONESb[:], BIH[:], start=False, stop=True
                    )
                    W2H = whp.tile([128, 512], BF16, tag="w2h", name="W2H")
                    nc.vector.tensor_copy(W2H[:], PS_w2[:])
                    return W2H

                def scale_b4(BW34):
                    # Fold diag(ch) into the streamed b4 tiles in place:
                    # (ctx . ch) @ b4T == ctx @ (diag(ch) b4T).
                    for c in range(NC):
                        nc.vector.tensor_scalar_mul(
                            BW34[:, NC + c, :], BW34[:, NC + c, :], CH[:, c : c + 1]
                        )

                beta_cur = fetch_beta(0)
                scale_b4(beta_cur[0])
                w2h_cur = produce_w2h(0, beta_cur[1], beta_cur[2])
                for h in range(NH):
                    BW34 = beta_cur[0]
                    beta_next = fetch_beta(h + 1) if h + 1 < NH else None
                    for t in range(NT):
                        PS_o = pj.tile([128, 512], F32, tag="o", name="PS_o")
                        for c in range(NC):
                            nc.tensor.matmul(
                                PS_o[:], B3T[:, c, ts(t, 128)], BW34[:, c, :],
                                start=(c == 0), stop=False,
                            )
                        for c in range(NC):
                            nc.tensor.matmul(
                                PS_o[:], CTt[t][:, c, :], BW34[:, NC + c, :],
                                start=False, stop=False,
                            )
                        nc.tensor.matmul(
                            PS_o[:], PnT[:, ts(t, 128)], w2h_cur[:],
                            start=False, stop=True,
                        )
                        OS = outp.tile([128, 512], F32, tag="os", name="OS")
                        if t % 2 == 0:
                            nc.vector.tensor_copy(OS[:], PS_o[:])
                        else:
                            nc.scalar.copy(OS[:], PS_o[:])
                        nc.sync.dma_start(d_out[ts(t, 128), ts(h, 512)], OS[:])
                    if beta_next is not None:
                        scale_b4(beta_next[0])
                        w2h_cur = produce_w2h(h + 1, beta_next[1], beta_next[2])
                        beta_cur = beta_next

    nc.compile()
    return nc


def _get_nc():
    global _NC_CACHE
    if _NC_CACHE is None:
        _NC_CACHE = _build()
    return _NC_CACHE


def _prep_shared(alpha_w, beta_w, beta_b):
    wc, wq, wcq = alpha_w[:D], alpha_w[D : 2 * D], alpha_w[2 * D :]
    shared = {
        "wcb8": np.ascontiguousarray(wc.reshape(NC, 128).T).astype(NPBF16),
        "wqb8": np.ascontiguousarray(wq.reshape(NC, 128).T).astype(NPBF16),
        "wcq8": np.ascontiguousarray(wcq.reshape(NC, 128).T).astype(np.float32),
        "identf": np.eye(128, dtype=np.float32),
        "bb": beta_b.reshape(1, H8).astype(NPBF16),
    }
    betaT = np.ascontiguousarray(beta_w.T)  # [f, o]
    # [h, p, c, f] per-partition contiguous packs of each block
    shared["b1h"] = np.ascontiguousarray(
        betaT[0:D].reshape(NC, 128, NH, 512).transpose(2, 1, 0, 3)
    ).astype(NPBF16)
    shared["b2h"] = np.ascontiguousarray(
        betaT[D : 2 * D].reshape(NC, 128, NH, 512).transpose(2, 1, 0, 3)
    ).astype(NPBF16)
    shared["b34h"] = np.ascontiguousarray(
        betaT[2 * D : 4 * D].reshape(2 * NC, 128, NH, 512).transpose(2, 1, 0, 3)
    ).astype(NPBF16)
    return shared


def kernel(context, query, alpha_w, alpha_b, beta_w, beta_b):
    global _LAST_EXEC_NS
    context = np.asarray(context, dtype=np.float32)
    query = np.asarray(query, dtype=np.float32)
    alpha_w = np.asarray(alpha_w, dtype=np.float32)
    beta_w = np.asarray(beta_w, dtype=np.float32)
    beta_b = np.asarray(beta_b, dtype=np.float32)

    shared = _prep_shared(alpha_w, beta_w, beta_b)

    in_maps = []
    for b in range(B):
        cb = context[b]
        qb = query[b]
        m = {
            # [t, ii, c, p] -> [t, p, c, ii]
            "ct8": np.ascontiguousarray(
                cb.reshape(NT, 128, NC, 128).transpose(0, 3, 2, 1)
            ).astype(NPBF16),
            # [t, ii, d] -> [ii, t, d]
            "cx8": np.ascontiguousarray(
                cb.reshape(NT, 128, D).transpose(1, 0, 2)
            ).astype(NPBF16),
            "q": qb.astype(NPBF16),
            # qT [d, j]: [c, p, j] -> [p, c, j]
            "qT8": np.ascontiguousarray(
                qb.T.reshape(NC, 128, QS).transpose(1, 0, 2)
            ).astype(NPBF16),
        }
        m.update(shared)
        in_maps.append(m)

    nc = _get_nc()
    res = run_bass_kernel_spmd(nc, in_maps, list(range(B)), trace=TRACE)
    _LAST_EXEC_NS = res.exec_time_ns
    out = np.stack([res.results[b]["out"] for b in range(B)], axis=0)
    return out
